# revision 16
# baseline (speedup 1.0000x reference)
"""Multi-head self-attention (16 heads, fake-quantized projections) on 8 trn2 cores.

Sharding: core c handles batch b = c // 4 and head group hg = c % 4 (global
heads 4*hg .. 4*hg+3). Each core computes its 4 heads' attention and a partial
output projection y^T [E, S]; the host sums the 4 partials per batch.

v2 design (fused pipeline, bf16 operands, multi-engine softmax):
  - All matmul operands bf16 (1 cyc/row on PE, half DMA/SBUF of fp32r).
  - Projection: x^T tiles stream in; q/k = W^T @ x^T in the interleaved
    [4h x d_lo | 4h x d_hi] layout; PSUM evacuated by ScalarE (Act Copy ->
    bf16), RoPE on DVE in 2x bf16 mode with GpSimd carrying half the chain;
    SBUF->SBUF DMA rearrange into head-contiguous q/k tiles (unchanged).
  - Attention steady loop over (qc, kt): packed score pairs (row-tiled 2
    heads/matmul), exp split between ScalarE (table exp) and DVE (Schraudolph
    bit-trick exp: int16(s*c1+c2) bitcast to bf16 ~ exp(s/8), ~3% PWL error,
    constant-factor part cancels in softmax), skewed PV (M=65 with ones row
    for the denominator), and the PREVIOUS q-chunk's epilogue interleaved:
    PSUM evict (DVE), reciprocal_approx_fast on the denominator row (DVE),
    row DMA to partition 0, partition_broadcast + normalize multiply
    (GpSimd), out-projection y^T = sum_h wo_h^T @ u_norm_h (PE), y eviction
    (ScalarE) and output DMA.
  - PSUM: 3 score banks + 4 PV banks + 1 y bank = 8 exactly.
Weights fake-quantized on host (exact numpy replica of the reference), all
device inputs bf16.
"""
import sys, types
import numpy as np

sys.path.insert(0, "/opt/trn_rl_repo")

# NTFF profile hook shim (stub antenv package lacks axon_hooks; harmless if absent)
try:
    from trn_agent_boot.trn_boot import _ntff_profile_via_ctypes
    _hook = _ntff_profile_via_ctypes("/opt/axon/libaxon_pjrt.so")
    _m = types.ModuleType("antenv.axon_hooks")
    _m.get_axon_ntff_profile_hook = lambda: _hook
    _m.set_axon_ntff_profile_hook = lambda h: None
    sys.modules.setdefault("antenv.axon_hooks", _m)
except Exception:
    pass

import ml_dtypes
import concourse.bacc as bacc
import concourse.tile as tile
from concourse import mybir
from concourse import bass_utils as _bu
from concourse.tile_rust import add_dep_helper
_bu.upload_artifacts = lambda tmpdir: "local://" + tmpdir

F32 = mybir.dt.float32
BF16 = mybir.dt.bfloat16
I16 = mybir.dt.int16
AF = mybir.ActivationFunctionType
ALU = mybir.AluOpType

B, S, E = 2, 2048, 1024
H, D = 16, 64
HL = 4          # heads per core
ET = E // 128   # 8 e-tiles
ST = S // 128   # 16 s-tiles
KT = S // 128   # 16 kpos tiles
SH = 1024       # s processed in halves in the projection phase
NQ = S // 512   # 4 q-chunks

# Schraudolph bf16 exp constants: i16 = trunc(s*C1 + C2); bitcast -> bf16
# approximates exp(s/8).  C1 = 128*log2(e)/8; C2 = 128*(127 - sigma) + 0.5
# (truncation centering; any constant factor cancels in the softmax ratio).
EXP_C1 = 128.0 * 1.4426950408889634 / 8.0
EXP_C2 = 128.0 * (127.0 - 0.0430) + 0.5


def quantize_bits_np(x):
    """Exact numpy replica of reference.quantize_bits(x, 8) in float32."""
    x = np.asarray(x, dtype=np.float32)
    qmax = np.float32(255.0)
    x_min = x.min()
    x_max = x.max()
    scale = np.float32((x_max - x_min) / np.float32(qmax + np.float32(1e-8)))
    x_q = np.round(np.clip((x - x_min) / np.float32(scale + np.float32(1e-8)),
                           np.float32(0.0), qmax)).astype(np.float32)
    return x_q * scale + x_min


def rope_tables():
    inv_freq = (1.0 / 10000.0 ** (np.arange(0, D, 2, dtype=np.float32) / D)).astype(np.float32)
    t = np.arange(S, dtype=np.float32)
    freqs = t[:, None].astype(np.float32) * inv_freq[None, :]
    sin = np.sin(freqs).astype(np.float32)   # (S, 32)
    cos = np.cos(freqs).astype(np.float32)
    cosT = np.tile(np.ascontiguousarray(cos.T), (4, 1))  # (128, S), [d, s]
    sinT = np.tile(np.ascontiguousarray(sin.T), (4, 1))
    return cosT, sinT


def build_kernel(debug=False):
    nc = bacc.Bacc(trn_type="TRN2")
    dbg = {}
    if debug:
        for name, shape, dt in [
                ("d_rot1", [128, SH], BF16),   # gpsimd rope out (sh0, q)
                ("d_p_act", [128, 512], BF16),  # Act exp p (qc0,kt0,pr0,j0)
                ("d_p_dve", [128, 512], BF16),  # DVE schraudolph p (qc0,kt0,pr0,j1)
                ("d_rr", [1, 512], F32),        # recip of denom (qc0,h0)
                ("d_bc", [D, 512], F32),        # broadcast recip
                ("d_uraw", [D, 512], BF16),     # u_raw (qc0,h0)
                ("d_un", [D, 512], BF16)]:      # u_norm[0][:, qc0]
            dbg[name] = nc.declare_dram_parameter(name, shape, dt, isOutput=True)
    xt = nc.declare_dram_parameter("xt", [E, S], BF16, isOutput=False)
    wqk = nc.declare_dram_parameter("wqk", [4, E, 128], BF16, isOutput=False)
    wv = nc.declare_dram_parameter("wv", [E, HL * D], BF16, isOutput=False)
    wout = nc.declare_dram_parameter("wout", [HL, D, E], BF16, isOutput=False)
    cost = nc.declare_dram_parameter("cost", [128, S], BF16, isOutput=False)
    sint = nc.declare_dram_parameter("sint", [128, S], BF16, isOutput=False)
    ypart = nc.declare_dram_parameter("ypart", [E, S], BF16, isOutput=True)

    with tile.TileContext(nc) as tc:
        with (
            tc.tile_pool(name="sb", bufs=1) as sb,
            tc.tile_pool(name="ps", bufs=2, space="PSUM") as ps,
        ):
            # ---------------- constants / weights
            cos_sb = sb.tile([128, S], BF16, tag="cs", bufs=2)
            sin_sb = sb.tile([128, S], BF16, tag="cs", bufs=2)
            nc.sync.dma_start(out=cos_sb, in_=cost[:, :])
            nc.sync.dma_start(out=sin_sb, in_=sint[:, :])
            wqk_sb = {}
            for ct in range(4):
                for et in range(ET):
                    t = sb.tile([128, 128], BF16, tag="wqk", bufs=32,
                                name=f"wqk{ct}_{et}")
                    nc.sync.dma_start(out=t, in_=wqk[ct, et * 128:(et + 1) * 128, :])
                    wqk_sb[(ct, et)] = t
            wv_sb = []
            for et in range(ET):
                t = sb.tile([128, HL * D], BF16, tag="wv", bufs=8, name=f"wv{et}")
                nc.sync.dma_start(out=t, in_=wv[et * 128:(et + 1) * 128, :])
                wv_sb.append(t)
            wo_sb = []
            for h in range(HL):
                t = sb.tile([D, E], BF16, tag="wo", bufs=4, name=f"wo{h}")
                nc.sync.dma_start(out=t, in_=wout[h, :, :])
                wo_sb.append(t)

            # prewarm the Exp activation table so the first attention exp
            # doesn't pay the table-load latency mid-pipeline
            warm = sb.tile([1, 8], F32, tag="warm", bufs=1)
            nc.vector.memset(warm, 0.0)
            warm2 = sb.tile([1, 8], BF16, tag="warm2", bufs=1)
            nc.scalar.activation(warm2, warm, AF.Exp, scale=0.125)

            # head-contiguous rotated q/k
            qcont = [sb.tile([128, S], BF16, tag="cont", bufs=4, name=f"qcont{p}")
                     for p in range(2)]
            kcont = [sb.tile([128, S], BF16, tag="cont", bufs=4, name=f"kcont{p}")
                     for p in range(2)]
            v_aug = []

            # ---------------- projection: q/k proj + RoPE + rearrange, v proj
            for sh in range(S // SH):
                ssl = slice(sh * SH, (sh + 1) * SH)
                xT = []
                for et in range(ET):
                    t = sb.tile([128, SH], BF16, tag="xT", bufs=ET, name=f"xT{et}")
                    nc.sync.dma_start(out=t, in_=xt[et * 128:(et + 1) * 128, ssl])
                    xT.append(t)

                for pair, dest in ((0, "q"), (2, "k")):
                    rot = [sb.tile([128, SH], BF16, tag="rot", bufs=4,
                                   name=f"rot{dest}{half}{sh}") for half in range(2)]
                    for q2 in range(SH // 512):
                        qsl = slice(q2 * 512, (q2 + 1) * 512)
                        gsl = slice(sh * SH + q2 * 512, sh * SH + (q2 + 1) * 512)
                        b1 = ps.tile([128, 512], F32, tag="sc", bufs=3)
                        b2 = ps.tile([128, 512], F32, tag="sc", bufs=3)
                        for et in range(ET):
                            nc.tensor.matmul(b1, wqk_sb[(pair, et)], xT[et][:, qsl],
                                             start=(et == 0), stop=(et == ET - 1))
                        for et in range(ET):
                            nc.tensor.matmul(b2, wqk_sb[(pair + 1, et)], xT[et][:, qsl],
                                             start=(et == 0), stop=(et == ET - 1))
                        # PSUM -> bf16 SBUF via ScalarE, freeing banks fast
                        r1 = sb.tile([128, 512], BF16, tag="r1", bufs=2)
                        r2 = sb.tile([128, 512], BF16, tag="r2", bufs=2)
                        nc.scalar.activation(r1, b1, AF.Copy)
                        nc.scalar.activation(r2, b2, AF.Copy)
                        # RoPE: DVE does the rot0 chain, GpSimd the rot1 chain
                        t1 = sb.tile([128, 512], BF16, tag="t1", bufs=2)
                        t2 = sb.tile([128, 512], BF16, tag="t2", bufs=2)
                        nc.vector.tensor_mul(t1, r1, cos_sb[:, gsl])
                        nc.vector.tensor_mul(t2, r2, sin_sb[:, gsl])
                        nc.vector.tensor_sub(rot[0][:, qsl], t1, t2)
                        t3 = sb.tile([128, 512], BF16, tag="t3", bufs=2)
                        t4 = sb.tile([128, 512], BF16, tag="t4", bufs=2)
                        nc.gpsimd.tensor_mul(t3, r1, sin_sb[:, gsl])
                        nc.gpsimd.tensor_mul(t4, r2, cos_sb[:, gsl])
                        nc.gpsimd.tensor_add(rot[1][:, qsl], t3, t4)
                    if debug and sh == 0 and pair == 0:
                        nc.sync.dma_start(out=dbg["d_rot1"][:, :], in_=rot[1])
                    cont = qcont if dest == "q" else kcont
                    for h in range(HL):
                        p, j = divmod(h, 2)
                        for half in range(2):
                            rows_out = slice(64 * j + 32 * half, 64 * j + 32 * half + 32)
                            nc.sync.dma_start(
                                out=cont[p][rows_out, ssl],
                                in_=rot[half][32 * h:32 * h + 32, :])

                # v projection (natural [s, d]) + ones column
                for st_l in range(SH // 128):
                    st = sh * (SH // 128) + st_l
                    pv = ps.tile([128, HL * D], F32, tag="sc", bufs=3)
                    for et in range(ET):
                        nc.tensor.matmul(pv, xT[et][:, st_l * 128:(st_l + 1) * 128],
                                         wv_sb[et], start=(et == 0), stop=(et == ET - 1))
                    va = sb.tile([128, HL, D + 1], BF16, tag="vaug", bufs=ST,
                                 name=f"vaug{st}")
                    nc.vector.memset(va, 1.0)
                    nc.vector.tensor_copy(va[:, :, 0:D],
                                          pv.rearrange("p (h d) -> p h d", h=HL))
                    v_aug.append(va)

            # ---------------- fused attention + epilogue/out-proj pipeline
            last_pe = None

            def pe_chain(mm):
                nonlocal last_pe
                if last_pe is not None:
                    add_dep_helper(mm.ins, last_pe.ins, sync=False, reason="pe order")
                last_pe = mm

            # exp engine schedule: per kt, 4 j-tiles; ScalarE gets 9/16,
            # DVE 7/16 of them.
            def exp_engine(kt, idx):
                if kt < 4:
                    # DVE is busy with the previous chunk's PSUM evictions/recips
                    return "A"
                pat = ("A", "D", "A", "D") if kt % 2 == 0 else ("A", "D", "A", "A")
                return pat[idx]

            u_norm = [sb.tile([D, S], BF16, tag="unorm", bufs=HL, name=f"unorm{h}")
                      for h in range(HL)]

            def emit_scores_exp(qc, kt, pr, p_out):
                """Scores for head pair pr at (qc, kt) + exp into p tiles."""
                qsl = slice(qc * 512, (qc + 1) * 512)
                for j in range(2):
                    s_ps = ps.tile([128, 512], F32, tag="sc", bufs=3)
                    mm = nc.tensor.matmul(
                        s_ps,
                        kcont[pr][64 * j:64 * j + 64, kt * 128:(kt + 1) * 128],
                        qcont[pr][64 * j:64 * j + 64, qsl],
                        start=True, stop=True)
                    pe_chain(mm)
                    p_t = sb.tile([128, 512], BF16, tag="p", bufs=8)
                    eng = exp_engine(kt, pr * 2 + j)
                    if eng == "A":
                        nc.scalar.activation(p_t, s_ps, AF.Exp, scale=0.125)
                    else:
                        with nc.allow_low_precision(reason="schraudolph exp"):
                            nc.vector.tensor_scalar(
                                p_t.bitcast(I16), s_ps, EXP_C1, EXP_C2,
                                op0=ALU.mult, op1=ALU.add)
                    if debug and qc == 0 and kt == 0 and pr == 0:
                        nc.sync.dma_start(
                            out=dbg["d_p_act" if j == 0 else "d_p_dve"][:, :], in_=p_t)
                    p_out[pr * 2 + j] = p_t

            def emit_pv(qc, kt, sup, p_ts, hs):
                for h in hs:
                    mm = nc.tensor.matmul(
                        sup[h], v_aug[kt][:, h, :], p_ts[h],
                        start=(kt == 0), stop=(kt == KT - 1))
                    pe_chain(mm)

            # epilogue state per finished q-chunk
            def emit_epilogue_slot(eqc, slot, st8):
                """One kt-slot worth of epilogue work for finished chunk eqc.
                st8: dict carrying per-eqc tiles across slots.
                Slot 0 evicts ALL four PSUM accumulators (so the next chunk's
                PV restart, emitted one kt later, never races the eviction)."""
                qsl = slice(eqc * 512, (eqc + 1) * 512)
                if slot == 0:
                    # Evict ALL four accumulators now: the next chunk's PV
                    # restart (one kt later) reuses these PSUM slots.
                    sup = st8["sup"]
                    for h in range(HL):
                        u_raw = sb.tile([D + 1, 512], BF16, tag="uraw", bufs=8,
                                        name=f"uraw{h}_{eqc}")
                        nc.vector.tensor_copy(u_raw, sup[h])
                        # denominator row -> partition 0 (custom-DVE recip is
                        # SBUF-only, and broadcast wants partition 0)
                        rbb = sb.tile([1, 512], BF16, tag="rbb", bufs=4)
                        nc.sync.dma_start(out=rbb, in_=u_raw[D:D + 1, :])
                        st8["u_raw"][h] = u_raw
                        st8["rbb"][h] = rbb
                elif slot == 1:
                    for h in range(HL):
                        rbf = sb.tile([1, 512], F32, tag="rbf", bufs=2)
                        nc.vector.tensor_copy(rbf, st8["rbb"][h])
                        rc = sb.tile([1, 512], F32, tag="rc", bufs=4)
                        with nc.allow_low_precision(reason="softmax denom recip"):
                            nc.vector.reciprocal_approx_fast(out=rc, in_=rbf)
                        st8["rc"][h] = rc
                elif slot == 2:
                    for h in range(HL):
                        bc = sb.tile([D, 512], F32, tag="bc", bufs=4)
                        nc.gpsimd.partition_broadcast(bc, st8["rc"][h])
                        nc.gpsimd.tensor_mul(u_norm[h][:, qsl],
                                             st8["u_raw"][h][0:D, :], bc)
                        if debug and eqc == 0 and h == 0:
                            nc.sync.dma_start(out=dbg["d_rr"][:, :], in_=st8["rc"][h])
                            nc.sync.dma_start(out=dbg["d_uraw"][:, :],
                                              in_=st8["u_raw"][h][0:D, :])
                            nc.sync.dma_start(out=dbg["d_bc"][:, :], in_=bc)
                            nc.sync.dma_start(out=dbg["d_un"][:, :],
                                              in_=u_norm[h][:, qsl])
                elif 7 <= slot < 11:
                    # y tiles late: ~5 kt of headroom over the normalize chain
                    # so these PE matmuls never block the in-order PE queue
                    for et in st8["ets"][(slot - 7) * 2:(slot - 7) * 2 + 2]:
                        y_ps = ps.tile([128, 512], F32, tag="y", bufs=1)
                        for h in range(HL):
                            mm = nc.tensor.matmul(
                                y_ps, wo_sb[h][:, et * 128:(et + 1) * 128],
                                u_norm[h][:, qsl],
                                start=(h == 0), stop=(h == HL - 1))
                            pe_chain(mm)
                        y_sb = sb.tile([128, 512], BF16, tag="ysb", bufs=2)
                        nc.scalar.activation(y_sb, y_ps, AF.Copy)
                        nc.sync.dma_start(out=ypart[et * 128:(et + 1) * 128, qsl],
                                          in_=y_sb)
                # other slots: idle

            # steady-state loop: skew PV one kt behind scores
            prev = None      # (qc, kt, sup, p_ts) awaiting PV
            epi = None       # epilogue state of the finished chunk
            for qc in range(NQ):
                sup = {h: ps.tile([D + 1, 512], F32, tag="pv", bufs=HL,
                                  name=f"u{h}_{qc}") for h in range(HL)}
                for kt in range(KT):
                    # epilogue of the finished chunk goes FIRST so its ops sit
                    # ahead of this kt's exp burst in the engine queues
                    if epi is not None and epi["slot"] < 16:
                        emit_epilogue_slot(epi["qc"], epi["slot"], epi)
                        epi["slot"] += 1
                    p_ts = {}
                    emit_scores_exp(qc, kt, 0, p_ts)
                    if prev is not None:
                        emit_pv(prev[0], prev[1], prev[2], prev[3], (0, 1))
                    emit_scores_exp(qc, kt, 1, p_ts)
                    if prev is not None:
                        emit_pv(prev[0], prev[1], prev[2], prev[3], (2, 3))
                        if prev[1] == KT - 1:
                            epi = {"qc": prev[0], "slot": 0, "sup": prev[2],
                                   "u_raw": {}, "rbb": {}, "rc": {},
                                   "ets": list(range(8))}
                    prev = (qc, kt, sup, p_ts)
            # drain: last PV, remaining epilogue slots, final chunk epilogue
            emit_pv(prev[0], prev[1], prev[2], prev[3], (0, 1, 2, 3))
            if epi is not None:
                while epi["slot"] < 16:
                    emit_epilogue_slot(epi["qc"], epi["slot"], epi)
                    epi["slot"] += 1
            epi = {"qc": prev[0], "slot": 0, "sup": prev[2],
                   "u_raw": {}, "rbb": {}, "rc": {}, "ets": list(range(8))}
            while epi["slot"] < 16:
                emit_epilogue_slot(epi["qc"], epi["slot"], epi)
                epi["slot"] += 1
    nc.finalize()
    return nc


def make_inputs(x, w_qkv, w_out):
    """Host-side prep: quantize, cast to bf16, split/re-layout per core."""
    x = np.asarray(x, dtype=np.float32)
    wq_deq = quantize_bits_np(np.asarray(w_qkv, dtype=np.float32))
    wo_deq = quantize_bits_np(np.asarray(w_out, dtype=np.float32))
    cosT, sinT = rope_tables()
    bf = ml_dtypes.bfloat16

    x_t = [np.ascontiguousarray(x[b].T).astype(bf) for b in range(B)]
    cos_b = cosT.astype(bf)
    sin_b = sinT.astype(bf)

    in_maps = []
    for c in range(8):
        b, hg = divmod(c, 4)
        heads = [hg * HL + i for i in range(HL)]
        # interleaved q/k col-tiles [4, E, 128]: 0=q d_lo, 1=q d_hi, 2=k d_lo, 3=k d_hi
        wqk_t = np.empty((4, E, 128), dtype=np.float32)
        for half in range(2):
            cols = np.concatenate(
                [np.arange(h * D + 32 * half, h * D + 32 * half + 32) for h in heads])
            wqk_t[0 + half] = wq_deq[:, 0 * E + cols]
            wqk_t[2 + half] = wq_deq[:, 1 * E + cols]
        vcols = np.concatenate([np.arange(h * D, h * D + D) for h in heads])
        wv_t = np.ascontiguousarray(wq_deq[:, 2 * E + vcols])
        wout_t = np.stack([wo_deq[h * D:(h + 1) * D, :] for h in heads])
        in_maps.append({
            "xt": x_t[b],
            "wqk": wqk_t.astype(bf), "wv": wv_t.astype(bf),
            "wout": wout_t.astype(bf),
            "cost": cos_b, "sint": sin_b,
        })
    return in_maps


_NC_CACHE = {}


def get_nc():
    if "nc" not in _NC_CACHE:
        _NC_CACHE["nc"] = build_kernel()
    return _NC_CACHE["nc"]


def kernel(x, w_qkv, w_out):
    from concourse.bass_utils import run_bass_kernel_spmd
    nc = get_nc()
    in_maps = make_inputs(x, w_qkv, w_out)
    res = run_bass_kernel_spmd(nc, in_maps, list(range(8)))
    out = np.zeros((B, S, E), dtype=np.float32)
    for c in range(8):
        out[c // 4] += np.asarray(res.results[c]["ypart"]).astype(np.float32).T
    return out
